# revision 64
# baseline (speedup 1.0000x reference)
"""Multi-head attention (B=2, S=2048, D=1024, H=16, Dh=64) on 8 Trainium2
NeuronCores via Bass/Tile.

Sharding: data-parallel over the 2 batches x tensor-parallel over head
groups (16 heads -> 4 groups of 4). Core c = 4*b + g handles batch b and
heads 4g..4g+3 with the matching column/row slices of Wq/Wk/Wv/Wo. Each
core returns its partial output projection (bf16); the host sums the 4
partials per batch in fp32 and adds bo' = bo + bv @ Wo (softmax weights
sum to 1, so the V bias commutes through Wo and never touches the device).

Host-side prep is layout only: inputs are pre-cast to bf16 and pre-tiled
into the exact SBUF layouts (x transposed to [D,S] and sliced) so every
load DMA moves large contiguous lines and no on-chip transposes exist.

Per-core kernel (4 heads = 2 pairs stacked on the 128-partition dim),
bf16 matmul datapath with fp32 PSUM accumulation:
  QT/KT = W^T x^T + b                  [128 (2 heads x 64), 2 pairs, S]
  V_ext = [maskf | 0-pad | (x Wv_g) * maskf]  per head, 128 cols
  per pair, per q-tile (512 queries), per key chunk (128 keys):
    scT [128k, 2x512q] = KT_chunk^T @ QT_tile  (2 heads row-packed in PE,
                                                concurrent via tile_position)
    eT  = exp(SCALE * scT)                     (one ACT op per chunk, bf16)
    ctx[128, 512] += V_ext_chunk^T @ eT_h      (row 0 = softmax denominator)
  normalize: recip(den @ psum partition 0) -> gpsimd broadcast -> ctx*rec
  out_partial = ctxT^T @ Wo_g            (PSUM accum over the 2 pairs)

Masked softmax: exp is taken over unmasked scores (|score*SCALE| small),
and the 0/1 key mask is folded into V_ext (zeroed V rows + mask column),
so masked keys contribute 0 to numerator and denominator -- no -inf.

The schedule is built around the ACT engine (exp is ACT-only at 1
elem/lane/cycle: ~147us busy vs ~137us of PE work), so the emission
order software-pipelines everything against it:
  - k_proj leads each chase block (only K gates the next score chunk);
    Q/V projections and the lagged ctx matmuls fill PE slack.
  - ctx matmuls trail their exp chunk by CTX_LAG chunks so, in the PE's
    in-order stream, upcoming score matmuls never queue behind
    ctx's wait-for-exp, and qt-boundary normalization is off-path.
  - each qt's first chunks are emitted before the previous qt's
    normalization (ctx PSUM slot handoff happens mid-stream).
  - the ACT exp table is preloaded via a dummy activation during DMA.
  - outputs stream out per 512-column half on both DMA queues; the last
    q-tile is normalized in 128-query slices with its out_proj matmuls
    and stores pipelined, and its evacuations ride the idle ACT engine.
"""

import ml_dtypes
import numpy as np

import concourse.bacc as bacc
import concourse.mybir as mybir
import concourse.tile as tile
from concourse.bass_utils import run_bass_kernel_spmd

F32 = mybir.dt.float32
BF16 = mybir.dt.bfloat16
AF = mybir.ActivationFunctionType

S = 2048
D = 1024
HPC = 4                  # heads per core
DH = 64
PAIRS = 2                # head pairs per core
P = 128
SC_CHUNKS = S // P       # 16 key chunks
QT_TILES = 4             # q tiles of 512
QW = 512                 # q tile width
ST_TILES = S // P        # 16 s tiles
DCH = D // P             # 8 D chunks
SCALE = 1.0 / np.sqrt(DH)
CTX_LAG = 7

N_CORES = 8


def build():
    nc = bacc.Bacc(None, target_bir_lowering=False, num_swdge_queues=4)

    # All inputs arrive host-pre-tiled in their exact SBUF layouts (bf16 where
    # the datapath is bf16) so every load DMA reads big contiguous lines.
    xt = nc.dram_tensor("xt", [P, QT_TILES, DCH, QW], BF16, kind="ExternalInput")
    wq0 = nc.dram_tensor("wq0", [P, DCH, P], BF16, kind="ExternalInput")
    wq1 = nc.dram_tensor("wq1", [P, DCH, P], BF16, kind="ExternalInput")
    wk0 = nc.dram_tensor("wk0", [P, DCH, P], BF16, kind="ExternalInput")
    wk1 = nc.dram_tensor("wk1", [P, DCH, P], BF16, kind="ExternalInput")
    wv = nc.dram_tensor("wv", [P, DCH, 256], BF16, kind="ExternalInput")
    wo = nc.dram_tensor("wo", [P, PAIRS, D], BF16, kind="ExternalInput")
    bq = nc.dram_tensor("bq", [P, PAIRS], F32, kind="ExternalInput")
    bk = nc.dram_tensor("bk", [P, PAIRS], F32, kind="ExternalInput")
    maskf = nc.dram_tensor("maskf", [P, SC_CHUNKS], F32, kind="ExternalInput")
    out = nc.dram_tensor("out", [S, D], BF16, kind="ExternalOutput")

    with tile.TileContext(nc) as tc:
        with (
            tc.tile_pool(name="persist", bufs=1) as pp,
            tc.tile_pool(name="expp", bufs=10) as ep,
            tc.tile_pool(name="ostage", bufs=2) as op_,
            tc.tile_pool(name="smalls", bufs=3) as sp,
            tc.tile_pool(name="ps_sc", bufs=2, space="PSUM") as ps_sc,
            tc.tile_pool(name="ps_ctx", bufs=2, space="PSUM") as ps_ctx,
            tc.tile_pool(name="ps_w", bufs=2, space="PSUM") as ps_w,
        ):
            # ---- constants / persistent tensors ----
            # weights first: qk_proj(0, 0) needs wq/wk asap
            wq_sb = pp.tile([P, DCH, 256], BF16)
            wk_sb = pp.tile([P, DCH, 256], BF16)
            wv_sb = pp.tile([P, DCH, 256], BF16)
            wo_sb = pp.tile([P, PAIRS, D], BF16)
            # casting DMAs (fp32 DRAM -> bf16 SBUF) must go via gpsimd/SWDGE
            # x slices stream on the HWDGE (sync) queue, weights on the SWDGE
            # (gpsimd) queues, so the qk chase and V projections are never
            # serialized behind each other's loads.
            bq_sb = pp.tile([P, PAIRS], F32)
            bk_sb = pp.tile([P, PAIRS], F32)
            maskp = pp.tile([P, SC_CHUNKS], F32)
            nc.sync.dma_start(maskp[:], maskf[:])
            nc.sync.dma_start(bq_sb[:], bq[:])
            nc.sync.dma_start(bk_sb[:], bk[:])
            # warm the ACT exp table while DMAs stream (table load ~2.7us)
            actwarm = sp.tile([P, PAIRS], F32, tag="actwarm")
            nc.scalar.activation(actwarm[:], bq_sb[:], AF.Exp, scale=1.0)
            # warm the ACT exp table while DMAs stream (table load ~2.7us)
            actwarm = sp.tile([P, PAIRS], F32, tag="actwarm")
            nc.scalar.activation(actwarm[:], bq_sb[:], AF.Exp, scale=1.0)
            # xT SBUF layout is slice-major so each half-slice load is fully
            # contiguous on both sides; halves split across the two queues.
            # Queue balance tuned to need-by time: each x slice is split in
            # dc-halves across the two queues; pair-1 QK weights and Wo are
            # only needed tens of us in, so they ride at the back.
            xT4 = pp.tile([P, QT_TILES, DCH, QW], BF16)
            HC = DCH // 2
            nc.sync.dma_start(wk_sb[:, :, 0:P], wk0[:])
            nc.gpsimd.dma_start(wq_sb[:, :, 0:P], wq0[:])
            nc.sync.dma_start(xT4[:, 0, 0:HC], xt[:, 0, 0:HC])
            nc.gpsimd.dma_start(xT4[:, 0, HC:DCH], xt[:, 0, HC:DCH])
            nc.sync.dma_start(xT4[:, 1, 0:HC], xt[:, 1, 0:HC])
            nc.gpsimd.dma_start(xT4[:, 1, HC:DCH], xt[:, 1, HC:DCH])
            nc.sync.dma_start(wv_sb[:], wv[:])
            nc.gpsimd.dma_start(xT4[:, 2, HC:DCH], xt[:, 2, HC:DCH])
            nc.sync.dma_start(xT4[:, 2, 0:HC], xt[:, 2, 0:HC])
            nc.gpsimd.dma_start(xT4[:, 3, HC:DCH], xt[:, 3, HC:DCH])
            nc.sync.dma_start(xT4[:, 3, 0:HC], xt[:, 3, 0:HC])
            nc.sync.dma_start(wq_sb[:, :, P:256], wq1[:])
            nc.sync.dma_start(wk_sb[:, :, P:256], wk1[:])
            nc.sync.dma_start(wo_sb[:], wo[:])

            # keep the PE busy during the DMA-bound lead-in so HAM is at
            # full clock (2.4GHz) when the first projection matmuls arrive
            warm = pp.tile([P, QW], BF16)
            nc.vector.memset(warm[:], 0.0)
            for _ in range(10):
                pw = ps_w.tile([P, QW], F32, tag="w")
                nc.tensor.matmul(pw[:], warm[:, 0:P], warm[:], start=True, stop=True)

            QT = pp.tile([P, PAIRS, S], BF16)
            KT = pp.tile([P, PAIRS, S], BF16)
            VE = pp.tile([P, SC_CHUNKS, HPC * P], BF16)
            ctxT = pp.tile([P, PAIRS, S], BF16)

            # mask columns of V_ext sit FIRST per head (so the softmax denom
            # lands on PSUM partition 0, readable by the fast-reciprocal
            # custom op directly); V columns follow at 1..DH
            ve4 = VE[:].rearrange("p st (h c) -> p st h c", h=HPC)
            nc.vector.tensor_copy(
                ve4[:, :, :, 0:1],
                maskp[:, :, None, None].to_broadcast([P, SC_CHUNKS, HPC, 1]),
            )

            def v_proj(st):
                j, off = st // 4, (st % 4) * P
                pv = ps_w.tile([P, QW], F32, tag="w")
                for dc in range(DCH):
                    nc.tensor.matmul(
                        pv[:, :256],
                        xT4[:, j, dc, off : off + P],
                        wv_sb[:, dc, :],
                        start=(dc == 0),
                        stop=(dc == DCH - 1),
                    )
                nc.vector.tensor_scalar_mul(
                    ve4[:, st, :, DH : 2 * DH],
                    pv[:, :256].rearrange("p (h c) -> p h c", h=HPC),
                    maskp[:, st : st + 1],
                )

            def _proj1(pr, qt, dst, w_sb, b_sb):
                sl = slice(qt * QW, (qt + 1) * QW)
                pq = ps_w.tile([P, QW], F32, tag="w")
                for dc in range(DCH):
                    nc.tensor.matmul(
                        pq[:],
                        w_sb[:, dc, pr * P : (pr + 1) * P],
                        xT4[:, qt, dc, :],
                        start=(dc == 0),
                        stop=(dc == DCH - 1),
                    )
                nc.vector.tensor_scalar_add(
                    dst[:, pr, sl], pq[:], b_sb[:, pr : pr + 1]
                )

            def k_proj(pr, qt):
                _proj1(pr, qt, KT, wk_sb, bk_sb)

            def q_proj(pr, qt):
                _proj1(pr, qt, QT, wq_sb, bq_sb)

            def qk_proj(pr, qt):
                k_proj(pr, qt)
                q_proj(pr, qt)

            att_state = {}

            def _emit_ctx(pr, qt, cps, et, kc):
                for hh in range(2):
                    h = 2 * pr + hh
                    nc.tensor.matmul(
                        cps[hh][:, :],
                        VE[:, kc, h * P : (h + 1) * P],
                        et[:, hh * QW : (hh + 1) * QW],
                        start=(kc == 0),
                        stop=(kc == SC_CHUNKS - 1),
                    )

            def attn_chunks(pr, qt, kcs, lag=None):
                lag = CTX_LAG if lag is None else lag
                qsl = slice(qt * QW, (qt + 1) * QW)
                if (pr, qt) not in att_state:
                    att_state[(pr, qt)] = [
                        [
                            ps_ctx.tile([P, QW], F32, tag="ctx", name=f"ctx{hh}")
                            for hh in range(2)
                        ],
                        [],  # pending (et, kc) whose ctx is not yet emitted
                    ]
                st_ = att_state[(pr, qt)]
                cps, pending = st_
                for kc in kcs:
                    sc = ps_sc.tile([P, 2 * QW], F32, tag="sc")
                    for hh in range(2):
                        nc.tensor.matmul(
                            sc[:, hh * QW : (hh + 1) * QW],
                            KT[hh * DH : (hh + 1) * DH, pr, kc * P : (kc + 1) * P],
                            QT[hh * DH : (hh + 1) * DH, pr, qsl],
                            start=True,
                            stop=True,
                            tile_position=(hh * DH, 0),
                        )
                    et = ep.tile([P, 2 * QW], BF16, tag="et")
                    nc.scalar.activation(et[:], sc[:], AF.Exp, scale=float(SCALE))
                    # ctx lags CTX_LAG chunks so upcoming scores aren't stuck
                    # behind ctx's wait-for-exp in the PE's in-order stream
                    # (also covers the previous qtile's normalization latency
                    # at qt boundaries before ctx's PSUM slot is reusable)
                    pending.append((et, kc))
                    while len(pending) > lag:
                        _emit_ctx(pr, qt, cps, *pending.pop(0))

            def attn_norm_fine(pr, qt):
                # last-tile tail: flush ctx, then normalize per 128-query
                # slice and issue each out_proj as soon as its slice is ready
                qsl0 = qt * QW
                cps, pending = att_state.pop((pr, qt))
                for p_ in pending:
                    _emit_ctx(pr, qt, cps, *p_)
                for i in range(4):
                    csl = slice(i * P, (i + 1) * P)
                    for hh in range(2):
                        denr = sp.tile([1, P], F32, tag="denrf", name=f"dnf{hh}")
                        nc.vector.reciprocal_approx_fast(
                            denr[:], cps[hh][0:1, csl]
                        )
                        recB = sp.tile([DH, P], F32, tag="recBf", name=f"rbf{hh}")
                        nc.gpsimd.partition_broadcast(recB[:], denr[:])
                        nc.vector.tensor_mul(
                            ctxT[hh * DH : (hh + 1) * DH, pr, qsl0 + i * P : qsl0 + (i + 1) * P],
                            cps[hh][DH : 2 * DH, csl],
                            recB[:],
                        )
                    out_proj(4 * qt + i)

            def attn_norm_fine(pr, qt):
                # last-tile tail: flush ctx, then normalize per 128-query
                # slice and issue each out_proj as soon as its slice is ready
                qsl0 = qt * QW
                cps, pending = att_state.pop((pr, qt))
                for p_ in pending:
                    _emit_ctx(pr, qt, cps, *p_)
                for i in range(4):
                    csl = slice(i * P, (i + 1) * P)
                    for hh in range(2):
                        denr = sp.tile([1, P], F32, tag="denrf", name=f"dnf{hh}")
                        nc.vector.reciprocal_approx_fast(
                            denr[:], cps[hh][0:1, csl]
                        )
                        recB = sp.tile([DH, P], F32, tag="recBf", name=f"rbf{hh}")
                        nc.gpsimd.partition_broadcast(recB[:], denr[:])
                        nc.vector.tensor_mul(
                            ctxT[hh * DH : (hh + 1) * DH, pr, qsl0 + i * P : qsl0 + (i + 1) * P],
                            cps[hh][DH : 2 * DH, csl],
                            recB[:],
                        )
                    out_proj(4 * qt + i)

            def attn_norm(pr, qt):
                qsl = slice(qt * QW, (qt + 1) * QW)
                cps, pending = att_state.pop((pr, qt))
                for p_ in pending:
                    _emit_ctx(pr, qt, cps, *p_)
                # normalize: recip(den@partition0) -> broadcast -> ctx*rec
                for hh in range(2):
                    denr = sp.tile([1, QW], F32, tag="denr", name=f"denr{hh}")
                    nc.vector.reciprocal_approx_fast(denr[:], cps[hh][0:1, :])
                    recB = sp.tile([DH, QW], F32, tag="recB", name=f"recB{hh}")
                    nc.gpsimd.partition_broadcast(recB[:], denr[:])
                    nc.vector.tensor_mul(
                        ctxT[hh * DH : (hh + 1) * DH, pr, qsl],
                        cps[hh][DH : 2 * DH, :],
                        recB[:],
                    )

            def out_proj(st):
                ob = op_.tile([P, D], BF16, tag="ob")
                for nt in range(2):
                    po = ps_w.tile([P, QW], F32, tag="w")
                    for pr in range(PAIRS):
                        nc.tensor.matmul(
                            po[:],
                            ctxT[:, pr, st * P : (st + 1) * P],
                            wo_sb[:, pr, nt * QW : (nt + 1) * QW],
                            start=(pr == 0),
                            stop=(pr == PAIRS - 1),
                        )
                    # the last stores' evacuations ride the (by then idle)
                    # ACT engine; mid-phase ones stay on DVE
                    if st >= 12:
                        nc.scalar.copy(ob[:, nt * QW : (nt + 1) * QW], po[:])
                    else:
                        nc.vector.tensor_copy(ob[:, nt * QW : (nt + 1) * QW], po[:])
                    # each half streams out as soon as its evacuation lands,
                    # split across both DMA queues to halve drain latency
                    eng = nc.sync if nt == 0 else nc.gpsimd
                    eng.dma_start(
                        out[st * P : (st + 1) * P, nt * QW : (nt + 1) * QW],
                        ob[:, nt * QW : (nt + 1) * QW],
                    )

            def attention(pr, qt):
                attn_chunks(pr, qt, range(SC_CHUNKS))
                attn_norm(pr, qt)

            # ---- emission order (sets scheduling priority) ----
            # Chase: attention(0,{0,1}) kc-chunks interleave with the pair-0
            # QK / V projection chase so exp starts early and has 2 qtiles of
            # runway per x slice; later projections (pair 1, out) are emitted
            # one step late so the next scores outrank them and ACT never
            # starves.
            # chase: only K(0, j) gates exp block j; each K is emitted one
            # chunk into the previous block (right when its x slice lands),
            # v_proj(st) rides right after chunk st+1 (ctx lags CTX_LAG), and
            # the pair-0 Q projections for qt 1..3 slot into leftover gaps.
            k_proj(0, 0)
            q_proj(0, 0)
            attn_chunks(0, 0, range(0, 2))
            k_proj(0, 1)
            v_proj(0)
            for kc in range(2, 16):
                attn_chunks(0, 0, range(kc, kc + 1))
                if kc in (5, 9):
                    k_proj(0, kc // 4 + 1)
                v_proj(kc - 1)
                if kc == 6:
                    q_proj(0, 1)
                if kc == 10:
                    q_proj(0, 2)
            # each qt's first 4 chunks (no ctx yet at lag 4) are emitted
            # BEFORE the previous qt's normalization, so boundary scores
            # never queue behind the norm chain in the in-order PE stream
            attn_chunks(0, 1, range(0, 4))
            v_proj(15)
            q_proj(0, 3)
            attn_norm(0, 0)
            attn_chunks(0, 1, range(4, 16))
            attn_chunks(0, 2, range(0, 4))
            k_proj(1, 0)
            attn_norm(0, 1)
            attn_chunks(0, 2, range(4, 10))
            q_proj(1, 0)
            attn_chunks(0, 2, range(10, 16))
            attn_chunks(0, 3, range(0, 4))
            k_proj(1, 1)
            attn_norm(0, 2)
            attn_chunks(0, 3, range(4, 10))
            k_proj(1, 2)
            attn_chunks(0, 3, range(10, 16))
            attn_chunks(1, 0, range(0, 4))
            q_proj(1, 1)
            attn_norm(0, 3)
            attn_chunks(1, 0, range(4, 8))
            k_proj(1, 3)
            attn_chunks(1, 0, range(8, 12))
            q_proj(1, 2)
            attn_chunks(1, 0, range(12, 16))
            q_proj(1, 3)
            attn_chunks(1, 1, range(0, 4))
            attn_norm(1, 0)
            attn_chunks(1, 1, range(4, 16))
            attn_chunks(1, 2, range(0, 4))
            attn_norm(1, 1)
            out_proj(0)
            out_proj(1)
            attn_chunks(1, 2, range(4, 16))
            out_proj(2)
            out_proj(3)
            attn_chunks(1, 3, range(0, 4))
            attn_norm(1, 2)
            out_proj(4)
            out_proj(5)
            attn_chunks(1, 3, range(4, 8))
            out_proj(6)
            out_proj(7)
            attn_chunks(1, 3, range(8, 12))
            out_proj(8)
            out_proj(9)
            # taper the ctx lag for the final tile: there are no later scores
            # to protect, and a short lag shrinks the post-last-exp flush
            attn_chunks(1, 3, range(12, 16), lag=2)
            out_proj(10)
            out_proj(11)
            attn_norm_fine(1, 3)

    nc.finalize()
    return nc


def ts(i, w):
    return slice(i * w, (i + 1) * w)


def _sb_w(w):
    """[D, n] weight slice -> SBUF layout [P, DCH, n]."""
    return np.ascontiguousarray(np.asarray(w).reshape(DCH, P, -1).transpose(1, 0, 2))


def shard_inputs(x, Wq, bq, Wk, bk, Wv, bv, Wo, bo, mask):
    """Full inputs -> list of 8 per-core input maps, pre-tiled to SBUF
    layouts (pure host-side layout prep; no kernel math moves to host)."""
    maskf = (~np.asarray(mask)).astype(np.float32)  # 1.0 = keep
    bf16 = ml_dtypes.bfloat16
    x = np.asarray(x, dtype=np.float32)
    # xt[p, j, c, s] = x[j*QW+s, c*P+p]
    xts = [
        np.ascontiguousarray(
            x[b].T.reshape(DCH, P, QT_TILES, QW).transpose(1, 2, 0, 3)
        ).astype(bf16)
        for b in range(2)
    ]
    Wqh = np.asarray(Wq, np.float32).astype(bf16)
    Wkh = np.asarray(Wk, np.float32).astype(bf16)
    Wvh = np.asarray(Wv, np.float32).astype(bf16)
    Woh = np.asarray(Wo, np.float32).astype(bf16)
    mask_t = [
        np.ascontiguousarray(maskf[b].reshape(SC_CHUNKS, P).T) for b in range(2)
    ]
    ins = []
    for c in range(N_CORES):
        b, g = divmod(c, 4)
        cs = slice(g * 256, (g + 1) * 256)
        wq_t = _sb_w(Wqh[:, cs])
        wk_t = _sb_w(Wkh[:, cs])
        ins.append(
            {
                "xt": xts[b],
                "wq0": np.ascontiguousarray(wq_t[:, :, 0:P]),
                "wq1": np.ascontiguousarray(wq_t[:, :, P:256]),
                "wk0": np.ascontiguousarray(wk_t[:, :, 0:P]),
                "wk1": np.ascontiguousarray(wk_t[:, :, P:256]),
                "wv": _sb_w(Wvh[:, cs]),
                "wo": np.ascontiguousarray(
                    Woh[cs, :].reshape(PAIRS, P, D).transpose(1, 0, 2)
                ),
                "bq": np.ascontiguousarray(
                    np.asarray(bq, np.float32)[cs].reshape(PAIRS, P).T
                ),
                "bk": np.ascontiguousarray(
                    np.asarray(bk, np.float32)[cs].reshape(PAIRS, P).T
                ),
                "maskf": mask_t[b],
            }
        )
    return ins


def gather_outputs(results, bv, Wo, bo):
    """8 per-core partial outputs -> full (2, S, D) fp32 output."""
    bo_eff = np.asarray(bo, dtype=np.float32) + np.asarray(
        bv, dtype=np.float32
    ) @ np.asarray(Wo, dtype=np.float32)
    outs = []
    for b in range(2):
        acc = results[4 * b]["out"].astype(np.float32).copy()
        for g in range(1, 4):
            acc += results[4 * b + g]["out"]
        outs.append(acc + bo_eff)
    return np.stack(outs, axis=0)


_NC_CACHE = []


def _get_nc():
    if not _NC_CACHE:
        _NC_CACHE.append(build())
    return _NC_CACHE[0]


def run_sharded(inputs, trace=False, tmpdir=None):
    """Shard, run on cores 0-7, gather. Returns (output, BassKernelResults)."""
    nc = _get_nc()
    ins = shard_inputs(**inputs)
    res = run_bass_kernel_spmd(
        nc, ins, core_ids=list(range(N_CORES)), trace=trace, tmpdir=tmpdir
    )
    full = gather_outputs(res.results, inputs["bv"], inputs["Wo"], inputs["bo"])
    return full, res


def kernel(**inputs) -> np.ndarray:
    full, _ = run_sharded(inputs, trace=False)
    return full


# revision 65
# speedup vs baseline: 1.0244x; 1.0244x over previous
"""Multi-head attention (B=2, S=2048, D=1024, H=16, Dh=64) on 8 Trainium2
NeuronCores via Bass/Tile.

Sharding: data-parallel over the 2 batches x tensor-parallel over head
groups (16 heads -> 4 groups of 4). Core c = 4*b + g handles batch b and
heads 4g..4g+3 with the matching column/row slices of Wq/Wk/Wv/Wo. Each
core returns its partial output projection (bf16); the host sums the 4
partials per batch in fp32 and adds bo' = bo + bv @ Wo (softmax weights
sum to 1, so the V bias commutes through Wo and never touches the device).

Host-side prep is layout only: inputs are pre-cast to bf16 and pre-tiled
into the exact SBUF layouts (x transposed to [D,S] and sliced) so every
load DMA moves large contiguous lines and no on-chip transposes exist.

Per-core kernel (4 heads = 2 pairs stacked on the 128-partition dim),
bf16 matmul datapath with fp32 PSUM accumulation:
  QT/KT = W^T x^T + b                  [128 (2 heads x 64), 2 pairs, S]
  V_ext = [maskf | 0-pad | (x Wv_g) * maskf]  per head, 128 cols
  per pair, per q-tile (512 queries), per key chunk (128 keys):
    scT [128k, 2x512q] = KT_chunk^T @ QT_tile  (2 heads row-packed in PE,
                                                concurrent via tile_position)
    eT  = exp(SCALE * scT)                     (one ACT op per chunk, bf16)
    ctx[128, 512] += V_ext_chunk^T @ eT_h      (row 0 = softmax denominator)
  normalize: recip(den @ psum partition 0) -> gpsimd broadcast -> ctx*rec
  out_partial = ctxT^T @ Wo_g            (PSUM accum over the 2 pairs)

Masked softmax: exp is taken over unmasked scores (|score*SCALE| small),
and the 0/1 key mask is folded into V_ext (zeroed V rows + mask column),
so masked keys contribute 0 to numerator and denominator -- no -inf.

The schedule is built around the ACT engine (exp is ACT-only at 1
elem/lane/cycle: ~147us busy vs ~137us of PE work), so the emission
order software-pipelines everything against it:
  - k_proj leads each chase block (only K gates the next score chunk);
    Q/V projections and the lagged ctx matmuls fill PE slack.
  - ctx matmuls trail their exp chunk by CTX_LAG chunks so, in the PE's
    in-order stream, upcoming score matmuls never queue behind
    ctx's wait-for-exp, and qt-boundary normalization is off-path.
  - each qt's first chunks are emitted before the previous qt's
    normalization (ctx PSUM slot handoff happens mid-stream).
  - the ACT exp table is preloaded via a dummy activation during DMA.
  - outputs stream out per 512-column half on both DMA queues; the last
    q-tile is normalized in 128-query slices with its out_proj matmuls
    and stores pipelined, and its evacuations ride the idle ACT engine.
"""

import ml_dtypes
import numpy as np

import concourse.bacc as bacc
import concourse.mybir as mybir
import concourse.tile as tile
from concourse.bass_utils import run_bass_kernel_spmd

F32 = mybir.dt.float32
BF16 = mybir.dt.bfloat16
AF = mybir.ActivationFunctionType

S = 2048
D = 1024
HPC = 4                  # heads per core
DH = 64
PAIRS = 2                # head pairs per core
P = 128
SC_CHUNKS = S // P       # 16 key chunks
QT_TILES = 4             # q tiles of 512
QW = 512                 # q tile width
ST_TILES = S // P        # 16 s tiles
DCH = D // P             # 8 D chunks
SCALE = 1.0 / np.sqrt(DH)
CTX_LAG = 7

N_CORES = 8


def build():
    nc = bacc.Bacc(None, target_bir_lowering=False, num_swdge_queues=4)

    # All inputs arrive host-pre-tiled in their exact SBUF layouts (bf16 where
    # the datapath is bf16) so every load DMA reads big contiguous lines.
    xt = nc.dram_tensor("xt", [P, QT_TILES, DCH, QW], BF16, kind="ExternalInput")
    wq0 = nc.dram_tensor("wq0", [P, DCH, P], BF16, kind="ExternalInput")
    wq1 = nc.dram_tensor("wq1", [P, DCH, P], BF16, kind="ExternalInput")
    wk0 = nc.dram_tensor("wk0", [P, DCH, P], BF16, kind="ExternalInput")
    wk1 = nc.dram_tensor("wk1", [P, DCH, P], BF16, kind="ExternalInput")
    wv = nc.dram_tensor("wv", [P, DCH, 256], BF16, kind="ExternalInput")
    wo = nc.dram_tensor("wo", [P, PAIRS, D], BF16, kind="ExternalInput")
    bq = nc.dram_tensor("bq", [P, PAIRS], F32, kind="ExternalInput")
    bk = nc.dram_tensor("bk", [P, PAIRS], F32, kind="ExternalInput")
    maskf = nc.dram_tensor("maskf", [P, SC_CHUNKS], F32, kind="ExternalInput")
    out = nc.dram_tensor("out", [S, D], BF16, kind="ExternalOutput")

    with tile.TileContext(nc) as tc:
        with (
            tc.tile_pool(name="persist", bufs=1) as pp,
            tc.tile_pool(name="expp", bufs=10) as ep,
            tc.tile_pool(name="ostage", bufs=2) as op_,
            tc.tile_pool(name="smalls", bufs=3) as sp,
            tc.tile_pool(name="ps_sc", bufs=2, space="PSUM") as ps_sc,
            tc.tile_pool(name="ps_ctx", bufs=2, space="PSUM") as ps_ctx,
            tc.tile_pool(name="ps_w", bufs=2, space="PSUM") as ps_w,
        ):
            # ---- constants / persistent tensors ----
            # weights first: qk_proj(0, 0) needs wq/wk asap
            wq_sb = pp.tile([P, DCH, 256], BF16)
            wk_sb = pp.tile([P, DCH, 256], BF16)
            wv_sb = pp.tile([P, DCH, 256], BF16)
            wo_sb = pp.tile([P, PAIRS, D], BF16)
            # casting DMAs (fp32 DRAM -> bf16 SBUF) must go via gpsimd/SWDGE
            # x slices stream on the HWDGE (sync) queue, weights on the SWDGE
            # (gpsimd) queues, so the qk chase and V projections are never
            # serialized behind each other's loads.
            bq_sb = pp.tile([P, PAIRS], F32)
            bk_sb = pp.tile([P, PAIRS], F32)
            maskp = pp.tile([P, SC_CHUNKS], F32)
            nc.sync.dma_start(maskp[:], maskf[:])
            nc.sync.dma_start(bq_sb[:], bq[:])
            nc.sync.dma_start(bk_sb[:], bk[:])
            # warm the ACT exp table while DMAs stream (table load ~2.7us)
            actwarm = sp.tile([P, PAIRS], F32, tag="actwarm")
            nc.scalar.activation(actwarm[:], bq_sb[:], AF.Exp, scale=1.0)
            # warm the ACT exp table while DMAs stream (table load ~2.7us)
            actwarm = sp.tile([P, PAIRS], F32, tag="actwarm")
            nc.scalar.activation(actwarm[:], bq_sb[:], AF.Exp, scale=1.0)
            # xT SBUF layout is slice-major so each half-slice load is fully
            # contiguous on both sides; halves split across the two queues.
            # Queue balance tuned to need-by time: each x slice is split in
            # dc-halves across the two queues; pair-1 QK weights and Wo are
            # only needed tens of us in, so they ride at the back.
            xT4 = pp.tile([P, QT_TILES, DCH, QW], BF16)
            HC = DCH // 2
            nc.sync.dma_start(xT4[:, 0, 0:HC], xt[:, 0, 0:HC])
            nc.gpsimd.dma_start(wq_sb[:, :, 0:P], wq0[:])
            nc.gpsimd.dma_start(wk_sb[:, :, 0:P], wk0[:])
            nc.gpsimd.dma_start(xT4[:, 0, HC:DCH], xt[:, 0, HC:DCH])
            nc.sync.dma_start(xT4[:, 1, 0:HC], xt[:, 1, 0:HC])
            nc.gpsimd.dma_start(xT4[:, 1, HC:DCH], xt[:, 1, HC:DCH])
            nc.sync.dma_start(wv_sb[:], wv[:])
            nc.gpsimd.dma_start(xT4[:, 2, HC:DCH], xt[:, 2, HC:DCH])
            nc.sync.dma_start(xT4[:, 2, 0:HC], xt[:, 2, 0:HC])
            nc.gpsimd.dma_start(xT4[:, 3, HC:DCH], xt[:, 3, HC:DCH])
            nc.sync.dma_start(xT4[:, 3, 0:HC], xt[:, 3, 0:HC])
            nc.sync.dma_start(wq_sb[:, :, P:256], wq1[:])
            nc.sync.dma_start(wk_sb[:, :, P:256], wk1[:])
            nc.sync.dma_start(wo_sb[:], wo[:])

            QT = pp.tile([P, PAIRS, S], BF16)
            KT = pp.tile([P, PAIRS, S], BF16)
            VE = pp.tile([P, SC_CHUNKS, HPC * P], BF16)
            ctxT = pp.tile([P, PAIRS, S], BF16)

            # mask columns of V_ext sit FIRST per head (so the softmax denom
            # lands on PSUM partition 0, readable by the fast-reciprocal
            # custom op directly); V columns follow at 1..DH
            ve4 = VE[:].rearrange("p st (h c) -> p st h c", h=HPC)
            nc.vector.tensor_copy(
                ve4[:, :, :, 0:1],
                maskp[:, :, None, None].to_broadcast([P, SC_CHUNKS, HPC, 1]),
            )

            def v_proj(st):
                j, off = st // 4, (st % 4) * P
                pv = ps_w.tile([P, QW], F32, tag="w")
                for dc in range(DCH):
                    nc.tensor.matmul(
                        pv[:, :256],
                        xT4[:, j, dc, off : off + P],
                        wv_sb[:, dc, :],
                        start=(dc == 0),
                        stop=(dc == DCH - 1),
                    )
                nc.vector.tensor_scalar_mul(
                    ve4[:, st, :, DH : 2 * DH],
                    pv[:, :256].rearrange("p (h c) -> p h c", h=HPC),
                    maskp[:, st : st + 1],
                )

            def _proj1(pr, qt, dst, w_sb, b_sb):
                sl = slice(qt * QW, (qt + 1) * QW)
                pq = ps_w.tile([P, QW], F32, tag="w")
                for dc in range(DCH):
                    nc.tensor.matmul(
                        pq[:],
                        w_sb[:, dc, pr * P : (pr + 1) * P],
                        xT4[:, qt, dc, :],
                        start=(dc == 0),
                        stop=(dc == DCH - 1),
                    )
                nc.vector.tensor_scalar_add(
                    dst[:, pr, sl], pq[:], b_sb[:, pr : pr + 1]
                )

            def k_proj(pr, qt):
                _proj1(pr, qt, KT, wk_sb, bk_sb)

            def q_proj(pr, qt):
                _proj1(pr, qt, QT, wq_sb, bq_sb)

            def qk_proj(pr, qt):
                k_proj(pr, qt)
                q_proj(pr, qt)

            att_state = {}

            def _emit_ctx(pr, qt, cps, et, kc):
                for hh in range(2):
                    h = 2 * pr + hh
                    nc.tensor.matmul(
                        cps[hh][:, :],
                        VE[:, kc, h * P : (h + 1) * P],
                        et[:, hh * QW : (hh + 1) * QW],
                        start=(kc == 0),
                        stop=(kc == SC_CHUNKS - 1),
                    )

            def attn_chunks(pr, qt, kcs, lag=None):
                lag = CTX_LAG if lag is None else lag
                qsl = slice(qt * QW, (qt + 1) * QW)
                if (pr, qt) not in att_state:
                    att_state[(pr, qt)] = [
                        [
                            ps_ctx.tile([P, QW], F32, tag="ctx", name=f"ctx{hh}")
                            for hh in range(2)
                        ],
                        [],  # pending (et, kc) whose ctx is not yet emitted
                    ]
                st_ = att_state[(pr, qt)]
                cps, pending = st_
                for kc in kcs:
                    sc = ps_sc.tile([P, 2 * QW], F32, tag="sc")
                    for hh in range(2):
                        nc.tensor.matmul(
                            sc[:, hh * QW : (hh + 1) * QW],
                            KT[hh * DH : (hh + 1) * DH, pr, kc * P : (kc + 1) * P],
                            QT[hh * DH : (hh + 1) * DH, pr, qsl],
                            start=True,
                            stop=True,
                            tile_position=(hh * DH, 0),
                        )
                    et = ep.tile([P, 2 * QW], BF16, tag="et")
                    nc.scalar.activation(et[:], sc[:], AF.Exp, scale=float(SCALE))
                    # ctx lags CTX_LAG chunks so upcoming scores aren't stuck
                    # behind ctx's wait-for-exp in the PE's in-order stream
                    # (also covers the previous qtile's normalization latency
                    # at qt boundaries before ctx's PSUM slot is reusable)
                    pending.append((et, kc))
                    while len(pending) > lag:
                        _emit_ctx(pr, qt, cps, *pending.pop(0))

            def attn_norm_fine(pr, qt):
                # last-tile tail: flush ctx, then normalize per 128-query
                # slice and issue each out_proj as soon as its slice is ready
                qsl0 = qt * QW
                cps, pending = att_state.pop((pr, qt))
                for p_ in pending:
                    _emit_ctx(pr, qt, cps, *p_)
                for i in range(4):
                    csl = slice(i * P, (i + 1) * P)
                    for hh in range(2):
                        denr = sp.tile([1, P], F32, tag="denrf", name=f"dnf{hh}")
                        nc.vector.reciprocal_approx_fast(
                            denr[:], cps[hh][0:1, csl]
                        )
                        recB = sp.tile([DH, P], F32, tag="recBf", name=f"rbf{hh}")
                        nc.gpsimd.partition_broadcast(recB[:], denr[:])
                        nc.vector.tensor_mul(
                            ctxT[hh * DH : (hh + 1) * DH, pr, qsl0 + i * P : qsl0 + (i + 1) * P],
                            cps[hh][DH : 2 * DH, csl],
                            recB[:],
                        )
                    out_proj(4 * qt + i)

            def attn_norm_fine(pr, qt):
                # last-tile tail: flush ctx, then normalize per 128-query
                # slice and issue each out_proj as soon as its slice is ready
                qsl0 = qt * QW
                cps, pending = att_state.pop((pr, qt))
                for p_ in pending:
                    _emit_ctx(pr, qt, cps, *p_)
                for i in range(4):
                    csl = slice(i * P, (i + 1) * P)
                    for hh in range(2):
                        denr = sp.tile([1, P], F32, tag="denrf", name=f"dnf{hh}")
                        nc.vector.reciprocal_approx_fast(
                            denr[:], cps[hh][0:1, csl]
                        )
                        recB = sp.tile([DH, P], F32, tag="recBf", name=f"rbf{hh}")
                        nc.gpsimd.partition_broadcast(recB[:], denr[:])
                        nc.vector.tensor_mul(
                            ctxT[hh * DH : (hh + 1) * DH, pr, qsl0 + i * P : qsl0 + (i + 1) * P],
                            cps[hh][DH : 2 * DH, csl],
                            recB[:],
                        )
                    out_proj(4 * qt + i)

            def attn_norm(pr, qt):
                qsl = slice(qt * QW, (qt + 1) * QW)
                cps, pending = att_state.pop((pr, qt))
                for p_ in pending:
                    _emit_ctx(pr, qt, cps, *p_)
                # normalize: recip(den@partition0) -> broadcast -> ctx*rec
                for hh in range(2):
                    denr = sp.tile([1, QW], F32, tag="denr", name=f"denr{hh}")
                    nc.vector.reciprocal_approx_fast(denr[:], cps[hh][0:1, :])
                    recB = sp.tile([DH, QW], F32, tag="recB", name=f"recB{hh}")
                    nc.gpsimd.partition_broadcast(recB[:], denr[:])
                    nc.vector.tensor_mul(
                        ctxT[hh * DH : (hh + 1) * DH, pr, qsl],
                        cps[hh][DH : 2 * DH, :],
                        recB[:],
                    )

            def out_proj(st):
                ob = op_.tile([P, D], BF16, tag="ob")
                for nt in range(2):
                    po = ps_w.tile([P, QW], F32, tag="w")
                    for pr in range(PAIRS):
                        nc.tensor.matmul(
                            po[:],
                            ctxT[:, pr, st * P : (st + 1) * P],
                            wo_sb[:, pr, nt * QW : (nt + 1) * QW],
                            start=(pr == 0),
                            stop=(pr == PAIRS - 1),
                        )
                    # the last stores' evacuations ride the (by then idle)
                    # ACT engine; mid-phase ones stay on DVE
                    if st >= 12:
                        nc.scalar.copy(ob[:, nt * QW : (nt + 1) * QW], po[:])
                    else:
                        nc.vector.tensor_copy(ob[:, nt * QW : (nt + 1) * QW], po[:])
                    # each half streams out as soon as its evacuation lands,
                    # split across both DMA queues to halve drain latency
                    eng = nc.sync if nt == 0 else nc.gpsimd
                    eng.dma_start(
                        out[st * P : (st + 1) * P, nt * QW : (nt + 1) * QW],
                        ob[:, nt * QW : (nt + 1) * QW],
                    )

            def attention(pr, qt):
                attn_chunks(pr, qt, range(SC_CHUNKS))
                attn_norm(pr, qt)

            # ---- emission order (sets scheduling priority) ----
            # Chase: attention(0,{0,1}) kc-chunks interleave with the pair-0
            # QK / V projection chase so exp starts early and has 2 qtiles of
            # runway per x slice; later projections (pair 1, out) are emitted
            # one step late so the next scores outrank them and ACT never
            # starves.
            # chase: only K(0, j) gates exp block j; each K is emitted one
            # chunk into the previous block (right when its x slice lands),
            # v_proj(st) rides right after chunk st+1 (ctx lags CTX_LAG), and
            # the pair-0 Q projections for qt 1..3 slot into leftover gaps.
            k_proj(0, 0)
            q_proj(0, 0)
            attn_chunks(0, 0, range(0, 2))
            k_proj(0, 1)
            v_proj(0)
            for kc in range(2, 16):
                attn_chunks(0, 0, range(kc, kc + 1))
                if kc in (5, 9):
                    k_proj(0, kc // 4 + 1)
                v_proj(kc - 1)
                if kc == 6:
                    q_proj(0, 1)
                if kc == 10:
                    q_proj(0, 2)
            # each qt's first 4 chunks (no ctx yet at lag 4) are emitted
            # BEFORE the previous qt's normalization, so boundary scores
            # never queue behind the norm chain in the in-order PE stream
            attn_chunks(0, 1, range(0, 4))
            v_proj(15)
            q_proj(0, 3)
            attn_norm(0, 0)
            attn_chunks(0, 1, range(4, 16))
            attn_chunks(0, 2, range(0, 4))
            k_proj(1, 0)
            attn_norm(0, 1)
            attn_chunks(0, 2, range(4, 10))
            q_proj(1, 0)
            attn_chunks(0, 2, range(10, 16))
            attn_chunks(0, 3, range(0, 4))
            k_proj(1, 1)
            attn_norm(0, 2)
            attn_chunks(0, 3, range(4, 10))
            k_proj(1, 2)
            attn_chunks(0, 3, range(10, 16))
            attn_chunks(1, 0, range(0, 4))
            q_proj(1, 1)
            attn_norm(0, 3)
            attn_chunks(1, 0, range(4, 8))
            k_proj(1, 3)
            attn_chunks(1, 0, range(8, 12))
            q_proj(1, 2)
            attn_chunks(1, 0, range(12, 16))
            q_proj(1, 3)
            attn_chunks(1, 1, range(0, 4))
            attn_norm(1, 0)
            attn_chunks(1, 1, range(4, 16))
            attn_chunks(1, 2, range(0, 4))
            attn_norm(1, 1)
            out_proj(0)
            out_proj(1)
            attn_chunks(1, 2, range(4, 16))
            out_proj(2)
            out_proj(3)
            attn_chunks(1, 3, range(0, 4))
            attn_norm(1, 2)
            out_proj(4)
            out_proj(5)
            attn_chunks(1, 3, range(4, 8))
            out_proj(6)
            out_proj(7)
            attn_chunks(1, 3, range(8, 12))
            out_proj(8)
            out_proj(9)
            # taper the ctx lag for the final tile: there are no later scores
            # to protect, and a short lag shrinks the post-last-exp flush
            attn_chunks(1, 3, range(12, 16), lag=2)
            out_proj(10)
            out_proj(11)
            attn_norm_fine(1, 3)

    nc.finalize()
    return nc


def ts(i, w):
    return slice(i * w, (i + 1) * w)


def _sb_w(w):
    """[D, n] weight slice -> SBUF layout [P, DCH, n]."""
    return np.ascontiguousarray(np.asarray(w).reshape(DCH, P, -1).transpose(1, 0, 2))


def shard_inputs(x, Wq, bq, Wk, bk, Wv, bv, Wo, bo, mask):
    """Full inputs -> list of 8 per-core input maps, pre-tiled to SBUF
    layouts (pure host-side layout prep; no kernel math moves to host)."""
    maskf = (~np.asarray(mask)).astype(np.float32)  # 1.0 = keep
    bf16 = ml_dtypes.bfloat16
    x = np.asarray(x, dtype=np.float32)
    # xt[p, j, c, s] = x[j*QW+s, c*P+p]
    xts = [
        np.ascontiguousarray(
            x[b].T.reshape(DCH, P, QT_TILES, QW).transpose(1, 2, 0, 3)
        ).astype(bf16)
        for b in range(2)
    ]
    Wqh = np.asarray(Wq, np.float32).astype(bf16)
    Wkh = np.asarray(Wk, np.float32).astype(bf16)
    Wvh = np.asarray(Wv, np.float32).astype(bf16)
    Woh = np.asarray(Wo, np.float32).astype(bf16)
    mask_t = [
        np.ascontiguousarray(maskf[b].reshape(SC_CHUNKS, P).T) for b in range(2)
    ]
    ins = []
    for c in range(N_CORES):
        b, g = divmod(c, 4)
        cs = slice(g * 256, (g + 1) * 256)
        wq_t = _sb_w(Wqh[:, cs])
        wk_t = _sb_w(Wkh[:, cs])
        ins.append(
            {
                "xt": xts[b],
                "wq0": np.ascontiguousarray(wq_t[:, :, 0:P]),
                "wq1": np.ascontiguousarray(wq_t[:, :, P:256]),
                "wk0": np.ascontiguousarray(wk_t[:, :, 0:P]),
                "wk1": np.ascontiguousarray(wk_t[:, :, P:256]),
                "wv": _sb_w(Wvh[:, cs]),
                "wo": np.ascontiguousarray(
                    Woh[cs, :].reshape(PAIRS, P, D).transpose(1, 0, 2)
                ),
                "bq": np.ascontiguousarray(
                    np.asarray(bq, np.float32)[cs].reshape(PAIRS, P).T
                ),
                "bk": np.ascontiguousarray(
                    np.asarray(bk, np.float32)[cs].reshape(PAIRS, P).T
                ),
                "maskf": mask_t[b],
            }
        )
    return ins


def gather_outputs(results, bv, Wo, bo):
    """8 per-core partial outputs -> full (2, S, D) fp32 output."""
    bo_eff = np.asarray(bo, dtype=np.float32) + np.asarray(
        bv, dtype=np.float32
    ) @ np.asarray(Wo, dtype=np.float32)
    outs = []
    for b in range(2):
        acc = results[4 * b]["out"].astype(np.float32).copy()
        for g in range(1, 4):
            acc += results[4 * b + g]["out"]
        outs.append(acc + bo_eff)
    return np.stack(outs, axis=0)


_NC_CACHE = []


def _get_nc():
    if not _NC_CACHE:
        _NC_CACHE.append(build())
    return _NC_CACHE[0]


def run_sharded(inputs, trace=False, tmpdir=None):
    """Shard, run on cores 0-7, gather. Returns (output, BassKernelResults)."""
    nc = _get_nc()
    ins = shard_inputs(**inputs)
    res = run_bass_kernel_spmd(
        nc, ins, core_ids=list(range(N_CORES)), trace=trace, tmpdir=tmpdir
    )
    full = gather_outputs(res.results, inputs["bv"], inputs["Wo"], inputs["bo"])
    return full, res


def kernel(**inputs) -> np.ndarray:
    full, _ = run_sharded(inputs, trace=False)
    return full
